# revision 9
# baseline (speedup 1.0000x reference)
"""DescriptorRetentionLoss on 8 Trainium2 cores.

Shards the N=4096 keypoint rows across 8 cores (512 rows each); memory
descriptors (M=8192) are replicated. The loss decomposes as

  loss = (S.T + A) / (max(n_pairs,1) * max(n_rows,1))
    S[m] = column match counts (summed over cores on host)
    T[m] = sum_n row_has[n] * cos[n, m]   (low-rank "w-trick": T = w @ yT',
           w = sum_n row_has[n]*rsqrt_xx[n]*x[n, :])
    A    = sum_n row_has[n] * (rowcount[n] - 2 * sum_m matchf*cos)

so each core emits only [M]-sized and [N/8]-sized partials; the final scalar
is assembled on host. cos is never materialized: the descriptor matmul PSUM is
consumed by a fused affine_mul_reduce against the match mask.

Two phases per core: phase 1 computes row match-counts only (so row_has and w
are available early); phase 2 recomputes the mask per column block and runs
the column counts, the big descriptor matmul, and the T matvec inline.
"""

import sys

sys.path.insert(0, "/opt/trn_rl_repo")

import numpy as np
from contextlib import ExitStack


def _split11(v):
    """Exact 2-piece split of fp32 into <=11-mantissa-bit halves."""
    v = np.asarray(v, np.float32)
    m, e = np.frexp(v)
    hi = np.ldexp(np.trunc(np.ldexp(m, 11)), e - 11).astype(np.float32)
    return hi, (v - hi).astype(np.float32)


def _split11_multi(v64, n):
    pieces = []
    rem = np.asarray(v64, np.float64)
    for _ in range(n):
        r32 = rem.astype(np.float32)
        m, e = np.frexp(r32)
        hi = np.ldexp(np.trunc(np.ldexp(m, 11)), e - 11).astype(np.float32)
        pieces.append(hi)
        rem = rem - hi.astype(np.float64)
    return pieces

N, M, D = 4096, 8192, 512
NCORES = 8
NL = N // NCORES          # 512 local rows per core
NT = NL // 128            # 4 n-tiles
MS = 16                   # m-subtiles
MSUB = M // MS            # 512
KC = D // 128             # 4 contraction chunks

_cached = {}


def _build_nc():
    from concourse import bacc, bass, mybir, tile

    f32 = mybir.dt.float32
    f32r = mybir.dt.float32r
    bf16 = mybir.dt.bfloat16
    f16 = mybir.dt.float16
    nc = bacc.Bacc("TRN2", target_bir_lowering=False, debug=False)

    xdT = nc.dram_tensor("xdT", [D, NL], bf16, kind="ExternalInput")
    x_nat = nc.dram_tensor("x_nat", [NL, D], f32, kind="ExternalInput")
    xpts = nc.dram_tensor("xpts", [11, NL], f32, kind="ExternalInput")
    thr = nc.dram_tensor("thr", [NL], f32, kind="ExternalInput")
    ypts = nc.dram_tensor("ypts", [11, M], f32, kind="ExternalInput")
    yT = nc.dram_tensor("yT", [D, M], bf16, kind="ExternalInput")

    S_out = nc.dram_tensor("S_out", [M], f32, kind="ExternalOutput")
    T_out = nc.dram_tensor("T_out", [M], f32, kind="ExternalOutput")
    ry_out = nc.dram_tensor("ry_out", [M], f32, kind="ExternalOutput")
    rc_out = nc.dram_tensor("rc_out", [NL], f32, kind="ExternalOutput")
    mc_out = nc.dram_tensor("mc_out", [NL], f32, kind="ExternalOutput")

    AF = mybir.ActivationFunctionType
    OP = mybir.AluOpType
    X = mybir.AxisListType.X

    with ExitStack() as ctx:
        tc = ctx.enter_context(tile.TileContext(nc))
        singles = ctx.enter_context(tc.tile_pool(name="singles", bufs=1))
        ytres_pool = ctx.enter_context(tc.tile_pool(name="ytres", bufs=MS))
        ryb_pool = ctx.enter_context(tc.tile_pool(name="ryb", bufs=1))
        sq_pool = ctx.enter_context(tc.tile_pool(name="sqp", bufs=4))
        mf_pool = ctx.enter_context(tc.tile_pool(name="mfp", bufs=6))
        mfs_pool = ctx.enter_context(tc.tile_pool(name="mfsp", bufs=6))
        small = ctx.enter_context(tc.tile_pool(name="small", bufs=8))
        evac_pool = ctx.enter_context(tc.tile_pool(name="evac", bufs=4))
        dram = ctx.enter_context(tc.tile_pool(name="dram", bufs=1, space="DRAM"))
        ps_p = ctx.enter_context(tc.tile_pool(name="ps_p", bufs=2, space="PSUM"))
        ps_d = ctx.enter_context(tc.tile_pool(name="ps_d", bufs=3, space="PSUM"))
        ps_s = ctx.enter_context(tc.tile_pool(name="ps_s", bufs=3, space="PSUM"))

        ones = singles.tile([128, 1], bf16)
        nc.vector.memset(ones, 1.0)
        ones_h = singles.tile([128, 1], f16)
        nc.vector.memset(ones_h, 1.0)

        # ---- x-side prep ----
        sxdT = singles.tile([128, KC, NL], bf16)
        nc.sync.dma_start(out=sxdT,
                          in_=xdT[:, :].rearrange("(c p) n -> p c n", p=128))
        sxn = singles.tile([128, NT, D], f32)
        nc.sync.dma_start(out=sxn,
                          in_=x_nat[:, :].rearrange("(t p) d -> p t d", p=128))
        sxpts = singles.tile([11, NL], f32r)
        nc.gpsimd.dma_start(out=sxpts, in_=xpts[:, :])
        sypts = singles.tile([11, M], f32r)
        nc.gpsimd.dma_start(out=sypts, in_=ypts[:, :])
        sthr = singles.tile([128, NT], f32)
        nc.sync.dma_start(out=sthr, in_=thr.rearrange("(t p) -> p t", p=128))

        rx = []
        for t in range(NT):
            xsq = sq_pool.tile([128, D], f16, name=f"xsq{t}", tag="sq")
            xxc = small.tile([128, 1], f32, name=f"xxc{t}", tag="stat1")
            nc.scalar.activation(xsq, sxn[:, t, :], AF.Square, accum_out=xxc)
            sx = small.tile([128, 1], f32, name=f"sx{t}", tag="stat1")
            nc.scalar.activation(sx, xxc, AF.Sqrt)
            rxt = singles.tile([128, 1], f32, name=f"rx{t}", tag=f"rx{t}")
            nc.vector.reciprocal(rxt, sx)
            rx.append(rxt)

        rcst = [singles.tile([128, MS], f32, name=f"rcst{t}", tag=f"rcst{t}")
                for t in range(NT)]
        mcst = [singles.tile([128, MS], f32, name=f"mcst{t}", tag=f"mcst{t}")
                for t in range(NT)]

        ry_dram = dram.tile([M], f32)
        ry_dram_pc = ry_dram[:].rearrange("(j t p) -> j p t", p=128, t=KC)
        yy_dram = dram.tile([M], f32)
        yy_dram_pc = yy_dram[:].rearrange("(j t p) -> j p t", p=128, t=KC)

        yt_res = []
        all_mfs = []
        rybs = []
        # ---- front-loaded: yT loads + column-norm chains for every block ----
        for j in range(MS):
            yt4 = ytres_pool.tile([128, KC, MSUB], bf16, name=f"yt{j}", tag="ytres")
            nc.sync.dma_start(
                out=yt4,
                in_=yT[:, j * MSUB:(j + 1) * MSUB].rearrange(
                    "(c p) m -> p c m", p=128))
            yt_res.append(yt4)

            p_yy = ps_s.tile([1, MSUB], f32, name=f"pyy{j}", tag="pS")
            for c in range(KC):
                ysq = sq_pool.tile([128, MSUB], f16, name=f"ysq{j}_{c}", tag="sq")
                nc.scalar.activation(ysq, yt4[:, c, :], AF.Square)
                nc.tensor.matmul(p_yy, ones_h, ysq, start=(c == 0),
                                 stop=(c == KC - 1))
            s_yy = evac_pool.tile([1, MSUB], f32, name=f"syyrow{j}", tag="sS")
            nc.scalar.activation(s_yy, p_yy, AF.Copy)
            nc.gpsimd.dma_start(out=yy_dram[j * MSUB:(j + 1) * MSUB], in_=s_yy)
            yyc = small.tile([128, KC], f32, name=f"yyc{j}", tag="yyc")
            nc.gpsimd.dma_start(out=yyc, in_=yy_dram_pc[j, :, :])
            syy = small.tile([128, KC], f32, name=f"syy{j}", tag="yyc")
            nc.scalar.activation(syy, yyc, AF.Sqrt)
            ryc = small.tile([128, KC], f32, name=f"ryc{j}", tag="yyc")
            nc.vector.reciprocal(ryc, syy)
            nc.gpsimd.dma_start(out=ry_dram_pc[j, :, :], in_=ryc)
            ryb = ryb_pool.tile([128, MSUB], f32, name=f"ryb{j}", tag=f"ryb{j}")
            seg = ry_dram[j * MSUB:(j + 1) * MSUB]
            bc_ap = bass.AP(tensor=seg.tensor, offset=seg.offset,
                            ap=[[0, 128], seg.ap[0]])
            nc.gpsimd.dma_start(out=ryb, in_=bc_ap)
            rybs.append(ryb)

        # ---- main compute pass over m-subtiles ----
        for j in range(MS):
            yt4 = yt_res[j]
            ryb = rybs[j]
            # pts matmul -> matchf (+row counts) -> S counts, ry-scaled mask
            mfs = []
            mfss = []
            for t in range(NT):
                pp = ps_p.tile([128, MSUB], f32, name=f"pp{j}_{t}", tag="pp")
                nc.tensor.matmul(pp, sxpts[:, t * 128:(t + 1) * 128],
                                 sypts[:, j * MSUB:(j + 1) * MSUB],
                                 start=True, stop=True)
                mf = mf_pool.tile([128, MSUB], bf16, name=f"mf{j}_{t}", tag="mf")
                nc.vector.tensor_scalar(
                    out=mf, in0=pp, scalar1=sthr[:, t:t + 1], scalar2=None,
                    op0=OP.is_lt, op1=OP.add,
                    accum_out=rcst[t][:, j:j + 1])
                mfs.append(mf)
                mfsc = mfs_pool.tile([128, MSUB], f32, name=f"mfs{j}_{t}", tag="mfs")
                nc.gpsimd.tensor_tensor(mfsc, mf, ryb, op=OP.mult)
                mfss.append(mfsc)
            all_mfs.append(mfs)
            pS = ps_s.tile([1, MSUB], f32, name=f"pS{j}", tag="pS")
            for t in range(NT):
                nc.tensor.matmul(pS, ones, mfs[t], start=(t == 0), stop=(t == NT - 1))
            sS = evac_pool.tile([1, MSUB], f32, name=f"sS{j}", tag="sS")
            nc.scalar.activation(sS, pS, AF.Copy)
            nc.sync.dma_start(out=S_out[j * MSUB:(j + 1) * MSUB], in_=sS)

            # descriptor matmul + fused mask-multiply-reduce
            for t in range(NT):
                pd = ps_d.tile([128, MSUB], f32, name=f"pd{j}_{t}", tag="pd")
                for c in range(KC):
                    nc.tensor.matmul(pd, sxdT[:, c, t * 128:(t + 1) * 128],
                                     yt4[:, c, :], start=(c == 0),
                                     stop=(c == KC - 1))
                dummy = small.tile([128, 1], f32, name=f"dm{j}_{t}", tag="dm")
                nc.vector.affine_mul_reduce(
                    out=dummy.broadcast_to(pd.shape),
                    accum_out=mcst[t][:, j:j + 1],
                    in0=pd, in1=mfss[t], scale=rx[t], bias=0.0)

        # ---- tail: rowcounts -> w -> T over resident yT ----
        gs = []
        rc_row = singles.tile([128, NT], f32)
        for t in range(NT):
            nc.vector.tensor_reduce(out=rc_row[:, t:t + 1], in_=rcst[t], axis=X,
                                    op=OP.add)
            rh = small.tile([128, 1], f32, name=f"rh{t}", tag="stat2")
            nc.vector.tensor_scalar(out=rh, in0=rc_row[:, t:t + 1], scalar1=0.0,
                                    scalar2=None, op0=OP.is_gt)
            g = singles.tile([128, 1], f32, name=f"g{t}", tag=f"g{t}")
            nc.vector.tensor_tensor(g, rh, rx[t], op=OP.mult)
            gs.append(g)
        nc.sync.dma_start(out=rc_out.rearrange("(t p) -> p t", p=128), in_=rc_row)

        w_dram = dram.tile([D], f32)
        pW = ps_s.tile([1, D], f32, name="pW", tag="pS")
        for t in range(NT):
            nc.tensor.matmul(pW, gs[t], sxn[:, t, :], start=(t == 0),
                             stop=(t == NT - 1))
        sW = evac_pool.tile([1, D], f32, name="sW", tag="sS")
        nc.scalar.activation(sW, pW, AF.Copy)
        nc.sync.dma_start(out=w_dram[:], in_=sW)
        wcol = singles.tile([128, KC], bf16)
        nc.gpsimd.dma_start(out=wcol, in_=w_dram[:].rearrange("(c p) -> p c", p=128))

        for j in range(MS):
            pT = ps_s.tile([1, MSUB], f32, name=f"pT{j}", tag="pS")
            for c in range(KC):
                nc.tensor.matmul(pT, wcol[:, c:c + 1], yt_res[j][:, c, :],
                                 start=(c == 0), stop=(c == KC - 1))
            sT = evac_pool.tile([1, MSUB], f32, name=f"sT{j}", tag="sS")
            nc.scalar.activation(sT, pT, AF.Copy)
            nc.sync.dma_start(out=T_out[j * MSUB:(j + 1) * MSUB], in_=sT)

        nc.sync.dma_start(out=ry_out[:], in_=ry_dram[:])

        # mc row sums
        mc_row = singles.tile([128, NT], f32)
        for t in range(NT):
            nc.vector.tensor_reduce(out=mc_row[:, t:t + 1], in_=mcst[t], axis=X,
                                    op=OP.add)
        nc.sync.dma_start(out=mc_out.rearrange("(t p) -> p t", p=128), in_=mc_row)

    nc.finalize()
    return nc


def _get_nc():
    if "nc" not in _cached:
        _cached["nc"] = _build_nc()
    return _cached["nc"]


def _mk_xpts(xp):
    x0h, x0l = _split11(xp[:, 0])
    x1h, x1l = _split11(xp[:, 1])
    one = np.ones(xp.shape[0], np.float32)
    # row k of xpts pairs with row k of ypts: [y0h,y0l,y0h,y0l,y1h,y1l,y1h,y1l,yy1,yy2,yy3]
    return np.ascontiguousarray(np.stack(
        [-2 * x0h, -2 * x0h, -2 * x0l, -2 * x0l,
         -2 * x1h, -2 * x1h, -2 * x1l, -2 * x1l, one, one, one]))


def _bf16np():
    import ml_dtypes
    return ml_dtypes.bfloat16


def _make_in_maps(valid_pts_scr, mem_pts_scr, valid_desc, mem_desc):
    y0h, y0l = _split11(mem_pts_scr[:, 0])
    y1h, y1l = _split11(mem_pts_scr[:, 1])
    yy64 = (mem_pts_scr[:, 0].astype(np.float64) ** 2
            + mem_pts_scr[:, 1].astype(np.float64) ** 2)
    yy1, yy2, yy3 = _split11_multi(yy64, 3)
    ypts = np.ascontiguousarray(
        np.stack([y0h, y0l, y0h, y0l, y1h, y1l, y1h, y1l, yy1, yy2, yy3]))
    import ml_dtypes
    yT = np.ascontiguousarray(mem_desc.T.astype(ml_dtypes.bfloat16))

    in_maps = []
    for c in range(NCORES):
        sl = slice(c * NL, (c + 1) * NL)
        xs = valid_desc[sl]
        xp = valid_pts_scr[sl]
        in_maps.append({
            "xdT": np.ascontiguousarray(xs.T.astype(_bf16np())),
            "x_nat": np.ascontiguousarray(xs),
            "xpts": _mk_xpts(xp),
            "thr": np.ascontiguousarray(
                (4.0 - xp[:, 0].astype(np.float64) ** 2
                 - xp[:, 1].astype(np.float64) ** 2).astype(np.float32)),
            "ypts": ypts,
            "yT": yT,
        })
    return in_maps


def _finish(results):
    S = np.zeros(M, np.float64)
    T = np.zeros(M, np.float64)
    A = 0.0
    nrows = 0.0
    for c in range(NCORES):
        r = results[c]
        S += r["S_out"].astype(np.float64)
        T += r["T_out"].astype(np.float64) * r["ry_out"].astype(np.float64)
        rc = r["rc_out"].astype(np.float64)
        mc = r["mc_out"].astype(np.float64)
        rh = rc > 0
        A += float(((rc - 2.0 * mc) * rh).sum())
        nrows += float(rh.sum())
    npairs = float(S.sum())
    if nrows > 0:
        loss = (float(S @ T) + A) / (max(npairs, 1.0) * max(nrows, 1.0))
    else:
        loss = 0.0
    return np.float32(loss)


def kernel(valid_pts_scr, mem_pts_scr, valid_desc, mem_desc):
    from concourse.bass_utils import run_bass_kernel_spmd

    in_maps = _make_in_maps(
        np.asarray(valid_pts_scr, dtype=np.float32),
        np.asarray(mem_pts_scr, dtype=np.float32),
        np.asarray(valid_desc, dtype=np.float32),
        np.asarray(mem_desc, dtype=np.float32))

    nc = _get_nc()
    res = run_bass_kernel_spmd(nc, in_maps, core_ids=list(range(NCORES)))
    _cached["last_results"] = res
    return _finish(res.results)


# revision 12
# speedup vs baseline: 53.6239x; 53.6239x over previous
"""DescriptorRetentionLoss on 8 Trainium2 cores.

Shards the N=4096 keypoint rows across 8 cores (512 rows each); memory
descriptors (M=8192) are replicated. The loss decomposes as

  loss = (S.T + A) / (max(n_pairs,1) * max(n_rows,1))
    S[m] = column match counts (summed over cores on host)
    T[m] = sum_n row_has[n] * cos[n, m]   (low-rank "w-trick": T = w @ yT',
           w = sum_n row_has[n]*rsqrt_xx[n]*x[n, :])
    A    = sum_n row_has[n] * (rowcount[n] - 2 * sum_m matchf*cos)

so each core emits only [M]-sized and [N/8]-sized partials; the final scalar
is assembled on host. cos is never materialized: the descriptor matmul PSUM is
consumed by a fused affine_mul_reduce against the match mask.

Two phases per core: phase 1 computes row match-counts only (so row_has and w
are available early); phase 2 recomputes the mask per column block and runs
the column counts, the big descriptor matmul, and the T matvec inline.
"""

import sys

sys.path.insert(0, "/opt/trn_rl_repo")

import numpy as np
from contextlib import ExitStack


def _split11(v):
    """Exact 2-piece split of fp32 into <=11-mantissa-bit halves."""
    v = np.asarray(v, np.float32)
    m, e = np.frexp(v)
    hi = np.ldexp(np.trunc(np.ldexp(m, 11)), e - 11).astype(np.float32)
    return hi, (v - hi).astype(np.float32)


def _split11_multi(v64, n):
    pieces = []
    rem = np.asarray(v64, np.float64)
    for _ in range(n):
        r32 = rem.astype(np.float32)
        m, e = np.frexp(r32)
        hi = np.ldexp(np.trunc(np.ldexp(m, 11)), e - 11).astype(np.float32)
        pieces.append(hi)
        rem = rem - hi.astype(np.float64)
    return pieces

N, M, D = 4096, 8192, 512
NCORES = 8
NL = N // NCORES          # 512 local rows per core
NT = NL // 128            # 4 n-tiles
MS = 16                   # m-subtiles
MSUB = M // MS            # 512
KC = D // 128             # 4 contraction chunks

_cached = {}


def _build_nc():
    from concourse import bacc, bass, mybir, tile

    f32 = mybir.dt.float32
    f32r = mybir.dt.float32r
    bf16 = mybir.dt.bfloat16
    f16 = mybir.dt.float16
    nc = bacc.Bacc("TRN2", target_bir_lowering=False, debug=False)

    xdT = nc.dram_tensor("xdT", [D, NL], bf16, kind="ExternalInput")
    x_nat = nc.dram_tensor("x_nat", [NL, D], f32, kind="ExternalInput")
    xpts = nc.dram_tensor("xpts", [11, NL], f32, kind="ExternalInput")
    thr = nc.dram_tensor("thr", [NL], f32, kind="ExternalInput")
    ypts = nc.dram_tensor("ypts", [11, M], f32, kind="ExternalInput")
    yT = nc.dram_tensor("yT", [D, M], bf16, kind="ExternalInput")

    S_out = nc.dram_tensor("S_out", [M], f32, kind="ExternalOutput")
    T_out = nc.dram_tensor("T_out", [M], f32, kind="ExternalOutput")
    ry_out = nc.dram_tensor("ry_out", [M], f32, kind="ExternalOutput")
    rc_out = nc.dram_tensor("rc_out", [NL], f32, kind="ExternalOutput")
    mc_out = nc.dram_tensor("mc_out", [NL], f32, kind="ExternalOutput")

    AF = mybir.ActivationFunctionType
    OP = mybir.AluOpType
    X = mybir.AxisListType.X

    with ExitStack() as ctx:
        tc = ctx.enter_context(tile.TileContext(nc))
        singles = ctx.enter_context(tc.tile_pool(name="singles", bufs=1))
        ytres_pool = ctx.enter_context(tc.tile_pool(name="ytres", bufs=MS))
        ryb_pool = ctx.enter_context(tc.tile_pool(name="ryb", bufs=1))
        sq_pool = ctx.enter_context(tc.tile_pool(name="sqp", bufs=4))
        mf_pool = ctx.enter_context(tc.tile_pool(name="mfp", bufs=6))
        mfs_pool = ctx.enter_context(tc.tile_pool(name="mfsp", bufs=6))
        small = ctx.enter_context(tc.tile_pool(name="small", bufs=8))
        evac_pool = ctx.enter_context(tc.tile_pool(name="evac", bufs=4))
        dram = ctx.enter_context(tc.tile_pool(name="dram", bufs=1, space="DRAM"))
        ps_p = ctx.enter_context(tc.tile_pool(name="ps_p", bufs=2, space="PSUM"))
        ps_d = ctx.enter_context(tc.tile_pool(name="ps_d", bufs=3, space="PSUM"))
        ps_s = ctx.enter_context(tc.tile_pool(name="ps_s", bufs=3, space="PSUM"))

        ones = singles.tile([128, 1], bf16)
        nc.vector.memset(ones, 1.0)
        ones_h = singles.tile([128, 1], f16)
        nc.vector.memset(ones_h, 1.0)

        # ---- x-side prep ----
        sxdT = singles.tile([128, KC, NL], bf16)
        nc.sync.dma_start(out=sxdT,
                          in_=xdT[:, :].rearrange("(c p) n -> p c n", p=128))
        sxn = singles.tile([128, NT, D], f32)
        nc.sync.dma_start(out=sxn,
                          in_=x_nat[:, :].rearrange("(t p) d -> p t d", p=128))
        sxpts = singles.tile([11, NL], f32r)
        nc.gpsimd.dma_start(out=sxpts, in_=xpts[:, :])
        sypts = singles.tile([11, M], f32r)
        for q in range(4):
            nc.gpsimd.dma_start(out=sypts[:, q * (M // 4):(q + 1) * (M // 4)],
                                in_=ypts[:, q * (M // 4):(q + 1) * (M // 4)])
        sthr = singles.tile([128, NT], f32)
        nc.sync.dma_start(out=sthr, in_=thr.rearrange("(t p) -> p t", p=128))

        rx = []
        for t in range(NT):
            xsq = sq_pool.tile([128, D], f16, name=f"xsq{t}", tag="sq")
            xxc = small.tile([128, 1], f32, name=f"xxc{t}", tag="stat1")
            nc.scalar.activation(xsq, sxn[:, t, :], AF.Square, accum_out=xxc)
            sx = small.tile([128, 1], f32, name=f"sx{t}", tag="stat1")
            nc.scalar.activation(sx, xxc, AF.Sqrt)
            rxt = singles.tile([128, 1], f32, name=f"rx{t}", tag=f"rx{t}")
            nc.vector.reciprocal(rxt, sx)
            rx.append(rxt)

        rcst = [singles.tile([128, MS], f32, name=f"rcst{t}", tag=f"rcst{t}")
                for t in range(NT)]
        mcst = [singles.tile([128, MS], f32, name=f"mcst{t}", tag=f"mcst{t}")
                for t in range(NT)]

        ry_dram = dram.tile([M], f32)
        ry_dram_pc = ry_dram[:].rearrange("(j t p) -> j p t", p=128, t=KC)
        yy_dram = dram.tile([M], f32)
        yy_dram_pc = yy_dram[:].rearrange("(j t p) -> j p t", p=128, t=KC)

        yt_res = []
        all_mfs = []
        rybs = []
        # ---- front-loaded: yT loads + column-norm chains for every block ----
        for j in range(MS):
            yt4 = ytres_pool.tile([128, KC, MSUB], bf16, name=f"yt{j}", tag="ytres")
            nc.sync.dma_start(
                out=yt4,
                in_=yT[:, j * MSUB:(j + 1) * MSUB].rearrange(
                    "(c p) m -> p c m", p=128))
            yt_res.append(yt4)

            p_yy = ps_s.tile([1, MSUB], f32, name=f"pyy{j}", tag="pS")
            for c in range(KC):
                ysq = sq_pool.tile([128, MSUB], f16, name=f"ysq{j}_{c}", tag="sq")
                nc.scalar.activation(ysq, yt4[:, c, :], AF.Square)
                nc.tensor.matmul(p_yy, ones_h, ysq, start=(c == 0),
                                 stop=(c == KC - 1))
            s_yy = evac_pool.tile([1, MSUB], f32, name=f"syyrow{j}", tag="sS")
            nc.scalar.activation(s_yy, p_yy, AF.Copy)
            nc.gpsimd.dma_start(out=yy_dram[j * MSUB:(j + 1) * MSUB], in_=s_yy)
            yyc = small.tile([128, KC], f32, name=f"yyc{j}", tag="yyc")
            nc.gpsimd.dma_start(out=yyc, in_=yy_dram_pc[j, :, :])
            syy = small.tile([128, KC], f32, name=f"syy{j}", tag="yyc")
            nc.scalar.activation(syy, yyc, AF.Sqrt)
            ryc = small.tile([128, KC], f32, name=f"ryc{j}", tag="yyc")
            nc.vector.reciprocal(ryc, syy)
            nc.gpsimd.dma_start(out=ry_dram_pc[j, :, :], in_=ryc)
            ryb = ryb_pool.tile([128, MSUB], f32, name=f"ryb{j}", tag=f"ryb{j}")
            seg = ry_dram[j * MSUB:(j + 1) * MSUB]
            bc_ap = bass.AP(tensor=seg.tensor, offset=seg.offset,
                            ap=[[0, 128], seg.ap[0]])
            nc.gpsimd.dma_start(out=ryb, in_=bc_ap)
            rybs.append(ryb)

        # ---- main compute pass over m-subtiles ----
        for j in range(MS):
            yt4 = yt_res[j]
            ryb = rybs[j]
            # pts matmul -> matchf (+row counts) -> S counts, ry-scaled mask
            mfs = []
            mfss = []
            for t in range(NT):
                pp = ps_p.tile([128, MSUB], f32, name=f"pp{j}_{t}", tag="pp")
                nc.tensor.matmul(pp, sxpts[:, t * 128:(t + 1) * 128],
                                 sypts[:, j * MSUB:(j + 1) * MSUB],
                                 start=True, stop=True)
                mf = mf_pool.tile([128, MSUB], bf16, name=f"mf{j}_{t}", tag="mf")
                nc.vector.tensor_scalar(
                    out=mf, in0=pp, scalar1=sthr[:, t:t + 1], scalar2=None,
                    op0=OP.is_lt, op1=OP.add,
                    accum_out=rcst[t][:, j:j + 1])
                mfs.append(mf)
                mfsc = mfs_pool.tile([128, MSUB], f32, name=f"mfs{j}_{t}", tag="mfs")
                nc.gpsimd.tensor_tensor(mfsc, mf, ryb, op=OP.mult)
                mfss.append(mfsc)
            all_mfs.append(mfs)
            pS = ps_s.tile([1, MSUB], f32, name=f"pS{j}", tag="pS")
            for t in range(NT):
                nc.tensor.matmul(pS, ones, mfs[t], start=(t == 0), stop=(t == NT - 1))
            sS = evac_pool.tile([1, MSUB], f32, name=f"sS{j}", tag="sS")
            nc.scalar.activation(sS, pS, AF.Copy)
            nc.sync.dma_start(out=S_out[j * MSUB:(j + 1) * MSUB], in_=sS)

            # descriptor matmul + fused mask-multiply-reduce
            for t in range(NT):
                pd = ps_d.tile([128, MSUB], f32, name=f"pd{j}_{t}", tag="pd")
                for c in range(KC):
                    nc.tensor.matmul(pd, sxdT[:, c, t * 128:(t + 1) * 128],
                                     yt4[:, c, :], start=(c == 0),
                                     stop=(c == KC - 1))
                dummy = small.tile([128, 1], f32, name=f"dm{j}_{t}", tag="dm")
                nc.vector.affine_mul_reduce(
                    out=dummy.broadcast_to(pd.shape),
                    accum_out=mcst[t][:, j:j + 1],
                    in0=pd, in1=mfss[t], scale=rx[t], bias=0.0)

        # ---- tail: rowcounts -> w -> T over resident yT ----
        gs = []
        rc_row = singles.tile([128, NT], f32)
        for t in range(NT):
            nc.vector.tensor_reduce(out=rc_row[:, t:t + 1], in_=rcst[t], axis=X,
                                    op=OP.add)
            rh = small.tile([128, 1], f32, name=f"rh{t}", tag="stat2")
            nc.vector.tensor_scalar(out=rh, in0=rc_row[:, t:t + 1], scalar1=0.0,
                                    scalar2=None, op0=OP.is_gt)
            g = singles.tile([128, 1], f32, name=f"g{t}", tag=f"g{t}")
            nc.vector.tensor_tensor(g, rh, rx[t], op=OP.mult)
            gs.append(g)
        nc.sync.dma_start(out=rc_out.rearrange("(t p) -> p t", p=128), in_=rc_row)

        w_dram = dram.tile([D], f32)
        pW = ps_s.tile([1, D], f32, name="pW", tag="pS")
        for t in range(NT):
            nc.tensor.matmul(pW, gs[t], sxn[:, t, :], start=(t == 0),
                             stop=(t == NT - 1))
        sW = evac_pool.tile([1, D], f32, name="sW", tag="sS")
        nc.scalar.activation(sW, pW, AF.Copy)
        nc.sync.dma_start(out=w_dram[:], in_=sW)
        wcol = singles.tile([128, KC], bf16)
        nc.gpsimd.dma_start(out=wcol, in_=w_dram[:].rearrange("(c p) -> p c", p=128))

        for j in range(MS):
            pT = ps_s.tile([1, MSUB], f32, name=f"pT{j}", tag="pS")
            for c in range(KC):
                nc.tensor.matmul(pT, wcol[:, c:c + 1], yt_res[j][:, c, :],
                                 start=(c == 0), stop=(c == KC - 1))
            sT = evac_pool.tile([1, MSUB], f32, name=f"sT{j}", tag="sS")
            nc.scalar.activation(sT, pT, AF.Copy)
            nc.sync.dma_start(out=T_out[j * MSUB:(j + 1) * MSUB], in_=sT)

        nc.sync.dma_start(out=ry_out[:], in_=ry_dram[:])

        # mc row sums
        mc_row = singles.tile([128, NT], f32)
        for t in range(NT):
            nc.vector.tensor_reduce(out=mc_row[:, t:t + 1], in_=mcst[t], axis=X,
                                    op=OP.add)
        nc.sync.dma_start(out=mc_out.rearrange("(t p) -> p t", p=128), in_=mc_row)

    nc.finalize()
    return nc


def _get_nc():
    if "nc" not in _cached:
        _cached["nc"] = _build_nc()
    return _cached["nc"]


def _mk_xpts(xp):
    x0h, x0l = _split11(xp[:, 0])
    x1h, x1l = _split11(xp[:, 1])
    one = np.ones(xp.shape[0], np.float32)
    # row k of xpts pairs with row k of ypts: [y0h,y0l,y0h,y0l,y1h,y1l,y1h,y1l,yy1,yy2,yy3]
    return np.ascontiguousarray(np.stack(
        [-2 * x0h, -2 * x0h, -2 * x0l, -2 * x0l,
         -2 * x1h, -2 * x1h, -2 * x1l, -2 * x1l, one, one, one]))


def _bf16np():
    import ml_dtypes
    return ml_dtypes.bfloat16


def _make_in_maps(valid_pts_scr, mem_pts_scr, valid_desc, mem_desc):
    y0h, y0l = _split11(mem_pts_scr[:, 0])
    y1h, y1l = _split11(mem_pts_scr[:, 1])
    yy64 = (mem_pts_scr[:, 0].astype(np.float64) ** 2
            + mem_pts_scr[:, 1].astype(np.float64) ** 2)
    yy1, yy2, yy3 = _split11_multi(yy64, 3)
    ypts = np.ascontiguousarray(
        np.stack([y0h, y0l, y0h, y0l, y1h, y1l, y1h, y1l, yy1, yy2, yy3]))
    import ml_dtypes
    yT = np.ascontiguousarray(mem_desc.T.astype(ml_dtypes.bfloat16))

    in_maps = []
    for c in range(NCORES):
        sl = slice(c * NL, (c + 1) * NL)
        xs = valid_desc[sl]
        xp = valid_pts_scr[sl]
        in_maps.append({
            "xdT": np.ascontiguousarray(xs.T.astype(_bf16np())),
            "x_nat": np.ascontiguousarray(xs),
            "xpts": _mk_xpts(xp),
            "thr": np.ascontiguousarray(
                (4.0 - xp[:, 0].astype(np.float64) ** 2
                 - xp[:, 1].astype(np.float64) ** 2).astype(np.float32)),
            "ypts": ypts,
            "yT": yT,
        })
    return in_maps


def _finish(results):
    S = np.zeros(M, np.float64)
    T = np.zeros(M, np.float64)
    A = 0.0
    nrows = 0.0
    for c in range(NCORES):
        r = results[c]
        S += r["S_out"].astype(np.float64)
        T += r["T_out"].astype(np.float64) * r["ry_out"].astype(np.float64)
        rc = r["rc_out"].astype(np.float64)
        mc = r["mc_out"].astype(np.float64)
        rh = rc > 0
        A += float(((rc - 2.0 * mc) * rh).sum())
        nrows += float(rh.sum())
    npairs = float(S.sum())
    if nrows > 0:
        loss = (float(S @ T) + A) / (max(npairs, 1.0) * max(nrows, 1.0))
    else:
        loss = 0.0
    return np.float32(loss)


def kernel(valid_pts_scr, mem_pts_scr, valid_desc, mem_desc):
    from concourse.bass_utils import run_bass_kernel_spmd

    in_maps = _make_in_maps(
        np.asarray(valid_pts_scr, dtype=np.float32),
        np.asarray(mem_pts_scr, dtype=np.float32),
        np.asarray(valid_desc, dtype=np.float32),
        np.asarray(mem_desc, dtype=np.float32))

    nc = _get_nc()
    res = run_bass_kernel_spmd(nc, in_maps, core_ids=list(range(NCORES)))
    _cached["last_results"] = res
    return _finish(res.results)


# revision 29
# speedup vs baseline: 64.9857x; 1.2119x over previous
"""DescriptorRetentionLoss on 8 Trainium2 cores.

Shards the N=4096 keypoint rows across 8 cores (512 rows each); memory
descriptors (M=8192) are replicated. The loss decomposes as

  loss = (S.T + A) / (max(n_pairs,1) * max(n_rows,1))
    S[m] = column match counts (summed over cores on host)
    T[m] = sum_n row_has[n] * cos[n, m]   (low-rank "w-trick": T = w @ yT',
           w = sum_n row_has[n]*rsqrt_xx[n]*x[n, :])
    A    = sum_n row_has[n] * (rowcount[n] - 2 * sum_m matchf*cos)

so each core emits only [M]-sized and [N/8]-sized partials; the final scalar
is assembled on host. cos is never materialized: the descriptor matmul PSUM is
consumed by a fused affine_mul_reduce against the match mask.

Two phases per core: phase 1 computes row match-counts only (so row_has and w
are available early); phase 2 recomputes the mask per column block and runs
the column counts, the big descriptor matmul, and the T matvec inline.
"""

import sys

sys.path.insert(0, "/opt/trn_rl_repo")

import numpy as np
from contextlib import ExitStack


def _split11(v):
    """Exact 2-piece split of fp32 into <=11-mantissa-bit halves."""
    v = np.asarray(v, np.float32)
    m, e = np.frexp(v)
    hi = np.ldexp(np.trunc(np.ldexp(m, 11)), e - 11).astype(np.float32)
    return hi, (v - hi).astype(np.float32)


def _split11_multi(v64, n):
    pieces = []
    rem = np.asarray(v64, np.float64)
    for _ in range(n):
        r32 = rem.astype(np.float32)
        m, e = np.frexp(r32)
        hi = np.ldexp(np.trunc(np.ldexp(m, 11)), e - 11).astype(np.float32)
        pieces.append(hi)
        rem = rem - hi.astype(np.float64)
    return pieces

N, M, D = 4096, 8192, 512
NCORES = 8
NL = N // NCORES          # 512 local rows per core
NT = NL // 128            # 4 n-tiles
MS = 16                   # m-subtiles
MSUB = M // MS            # 512
KC = D // 128             # 4 contraction chunks

_cached = {}


def _build_nc():
    from concourse import bacc, bass, mybir, tile

    f32 = mybir.dt.float32
    f32r = mybir.dt.float32r
    bf16 = mybir.dt.bfloat16
    f16 = mybir.dt.float16
    nc = bacc.Bacc("TRN2", target_bir_lowering=False, debug=False)

    xdT = nc.dram_tensor("xdT", [D, NL], bf16, kind="ExternalInput")
    x_nat = nc.dram_tensor("x_nat", [NL, D], f32, kind="ExternalInput")
    xpts = nc.dram_tensor("xpts", [11, NL], f32, kind="ExternalInput")
    thr = nc.dram_tensor("thr", [NL], f32, kind="ExternalInput")
    ypts = nc.dram_tensor("ypts", [11, M], f32, kind="ExternalInput")
    yT = nc.dram_tensor("yT", [D, M], bf16, kind="ExternalInput")

    S_out = nc.dram_tensor("S_out", [M], f32, kind="ExternalOutput")
    T_out = nc.dram_tensor("T_out", [M], f32, kind="ExternalOutput")
    ry_out = nc.dram_tensor("ry_out", [M], f32, kind="ExternalOutput")
    rc_out = nc.dram_tensor("rc_out", [NL], f32, kind="ExternalOutput")
    mc_out = nc.dram_tensor("mc_out", [NL], f32, kind="ExternalOutput")

    AF = mybir.ActivationFunctionType
    OP = mybir.AluOpType
    X = mybir.AxisListType.X

    with ExitStack() as ctx:
        tc = ctx.enter_context(tile.TileContext(nc))
        singles = ctx.enter_context(tc.tile_pool(name="singles", bufs=1))
        ytres_pool = ctx.enter_context(tc.tile_pool(name="ytres", bufs=MS))
        ryb_pool = ctx.enter_context(tc.tile_pool(name="ryb", bufs=6))
        yp_pool = ctx.enter_context(tc.tile_pool(name="ypp", bufs=4))
        sq_pool = ctx.enter_context(tc.tile_pool(name="sqp", bufs=6))
        mf_pool = ctx.enter_context(tc.tile_pool(name="mfp", bufs=10))
        mfs_pool = ctx.enter_context(tc.tile_pool(name="mfsp", bufs=1))
        small = ctx.enter_context(tc.tile_pool(name="small", bufs=10))
        evac_pool = ctx.enter_context(tc.tile_pool(name="evac", bufs=4))
        dram = ctx.enter_context(tc.tile_pool(name="dram", bufs=1, space="DRAM"))
        ps_p = ctx.enter_context(tc.tile_pool(name="ps_p", bufs=2, space="PSUM"))
        ps_d = ctx.enter_context(tc.tile_pool(name="ps_d", bufs=3, space="PSUM"))
        ps_s = ctx.enter_context(tc.tile_pool(name="ps_s", bufs=3, space="PSUM"))

        ones = singles.tile([128, 1], bf16)
        nc.vector.memset(ones, 1.0)
        ones_h = singles.tile([128, 1], f16)
        nc.vector.memset(ones_h, 1.0)

        # ---- x-side prep ----
        sxdT = singles.tile([128, KC, NL], bf16)
        nc.sync.dma_start(out=sxdT,
                          in_=xdT[:, :].rearrange("(c p) n -> p c n", p=128))
        sxn = singles.tile([128, NT, D], f32)
        nc.sync.dma_start(out=sxn,
                          in_=x_nat[:, :].rearrange("(t p) d -> p t d", p=128))
        sxpts = singles.tile([11, NL], f32r)
        nc.gpsimd.dma_start(out=sxpts, in_=xpts[:, :])
        ypts3 = ypts.rearrange("k (j f) -> k j f", f=MSUB)
        sthr = singles.tile([128, NT], f32)
        nc.sync.dma_start(out=sthr, in_=thr.rearrange("(t p) -> p t", p=128))

        rx = []
        for t in range(NT):
            xsq = sq_pool.tile([128, D], f16, name=f"xsq{t}", tag="sq")
            xxc = small.tile([128, 1], f32, name=f"xxc{t}", tag="stat1")
            nc.scalar.activation(xsq, sxn[:, t, :], AF.Square, accum_out=xxc)
            sx = small.tile([128, 1], f32, name=f"sx{t}", tag="stat1")
            nc.scalar.activation(sx, xxc, AF.Sqrt)
            rxt = singles.tile([128, 1], f32, name=f"rx{t}", tag=f"rx{t}")
            nc.vector.reciprocal(rxt, sx)
            rx.append(rxt)

        rcst = [singles.tile([128, MS], f32, name=f"rcst{t}", tag=f"rcst{t}")
                for t in range(NT)]
        mcst = [singles.tile([128, MS], f32, name=f"mcst{t}", tag=f"mcst{t}")
                for t in range(NT)]

        ry_dram = dram.tile([M], f32)
        ry_dram_pc = ry_dram[:].rearrange("(j t p) -> j p t", p=128, t=KC)
        yy_dram = dram.tile([M], f32)
        yy_dram_pc = yy_dram[:].rearrange("(j t p) -> j p t", p=128, t=KC)

        yt_res = []
        mfss_store = []
        # ---- loop A: yT loads, norm chains, mask pass (pts/TS/S/scaled mask) ----
        for j in range(MS):
            yt4 = ytres_pool.tile([128, KC, MSUB], bf16, name=f"yt{j}", tag="ytres")
            nc.sync.dma_start(
                out=yt4,
                in_=yT[:, j * MSUB:(j + 1) * MSUB].rearrange(
                    "(c p) m -> p c m", p=128))
            yt_res.append(yt4)

            p_yy = ps_s.tile([1, MSUB], f32, name=f"pyy{j}", tag="pS")
            ysq = sq_pool.tile([128, KC, MSUB], f16, name=f"ysq{j}", tag="sq")
            nc.scalar.activation(ysq, yt4[:, :, :], AF.Square)
            for c in range(KC):
                nc.tensor.matmul(p_yy, ones_h, ysq[:, c, :], start=(c == 0),
                                 stop=(c == KC - 1))
            s_yy = evac_pool.tile([1, MSUB], f32, name=f"syyrow{j}", tag="sS")
            nc.scalar.activation(s_yy, p_yy, AF.Copy)
            nc.gpsimd.dma_start(out=yy_dram[j * MSUB:(j + 1) * MSUB], in_=s_yy)
            yyc = small.tile([128, KC], f32, name=f"yyc{j}", tag="yyc")
            nc.gpsimd.dma_start(out=yyc, in_=yy_dram_pc[j, :, :])
            syy = small.tile([128, KC], f32, name=f"syy{j}", tag="yyc")
            nc.scalar.activation(syy, yyc, AF.Sqrt)
            ryc = small.tile([128, KC], f32, name=f"ryc{j}", tag="yyc")
            nc.vector.reciprocal(ryc, syy)
            nc.gpsimd.dma_start(out=ry_dram_pc[j, :, :], in_=ryc)
            ryb = ryb_pool.tile([128, MSUB], f32, name=f"ryb{j}", tag="ryb")
            seg = ry_dram[j * MSUB:(j + 1) * MSUB]
            bc_ap = bass.AP(tensor=seg.tensor, offset=seg.offset,
                            ap=[[0, 128], seg.ap[0]])
            nc.gpsimd.dma_start(out=ryb, in_=bc_ap)
            syp = yp_pool.tile([11, MSUB], f32r, name=f"syp{j}", tag="syp")
            nc.gpsimd.dma_start(out=syp, in_=ypts3[:, j, :])
            mfs_j = []
            mfl = []
            for t in range(NT):
                pp = ps_p.tile([128, MSUB], f32, name=f"pp{j}_{t}", tag="pp")
                nc.tensor.matmul(pp, sxpts[:, t * 128:(t + 1) * 128], syp,
                                 start=True, stop=True)
                mf = mf_pool.tile([128, MSUB], bf16, name=f"mf{j}_{t}", tag="mf")
                nc.vector.tensor_scalar(
                    out=mf, in0=pp, scalar1=sthr[:, t:t + 1], scalar2=None,
                    op0=OP.is_lt, op1=OP.add,
                    accum_out=rcst[t][:, j:j + 1])
                mfl.append(mf)
                mfsc = mfs_pool.tile([128, MSUB], bf16, name=f"mfs{j}_{t}",
                                     tag=f"mfs{j}_{t}")
                nc.gpsimd.tensor_tensor(mfsc, mf, ryb, op=OP.mult)
                mfs_j.append(mfsc)
            mfss_store.append(mfs_j)
            pS = ps_s.tile([1, MSUB], f32, name=f"pS{j}", tag="pS")
            for t in range(NT):
                nc.tensor.matmul(pS, ones, mfl[t], start=(t == 0), stop=(t == NT - 1))
            sS = evac_pool.tile([1, MSUB], f32, name=f"sS{j}", tag="sS")
            nc.scalar.activation(sS, pS, AF.Copy)
            nc.sync.dma_start(out=S_out[j * MSUB:(j + 1) * MSUB], in_=sS)


        # ---- w-chain (rowcounts complete after loop A) ----
        gs = []
        rc_row = singles.tile([128, NT], f32)
        for t in range(NT):
            nc.vector.tensor_reduce(out=rc_row[:, t:t + 1], in_=rcst[t], axis=X,
                                    op=OP.add)
            rh = small.tile([128, 1], f32, name=f"rh{t}", tag="stat2")
            nc.vector.tensor_scalar(out=rh, in0=rc_row[:, t:t + 1], scalar1=0.0,
                                    scalar2=None, op0=OP.is_gt)
            g = singles.tile([128, 1], f32, name=f"g{t}", tag=f"g{t}")
            nc.vector.tensor_tensor(g, rh, rx[t], op=OP.mult)
            gs.append(g)
        nc.sync.dma_start(out=rc_out.rearrange("(t p) -> p t", p=128), in_=rc_row)

        w_dram = dram.tile([D], f32)
        pW = ps_s.tile([1, D], f32, name="pW", tag="pS")
        for t in range(NT):
            nc.tensor.matmul(pW, gs[t], sxn[:, t, :], start=(t == 0),
                             stop=(t == NT - 1))
        sW = evac_pool.tile([1, D], f32, name="sW", tag="sS")
        nc.scalar.activation(sW, pW, AF.Copy)
        nc.sync.dma_start(out=w_dram[:], in_=sW)
        wcol = singles.tile([128, KC], bf16)
        nc.gpsimd.dma_start(out=wcol, in_=w_dram[:].rearrange("(c p) -> p c", p=128))

        # ---- loop B: descriptor matmuls + fused reduce + T matvec ----
        for j in range(MS):
            yt4 = yt_res[j]
            for t in range(NT):
                pd = ps_d.tile([128, MSUB], f32, name=f"pd{j}_{t}", tag="pd")
                for c in range(KC):
                    nc.tensor.matmul(pd, sxdT[:, c, t * 128:(t + 1) * 128],
                                     yt4[:, c, :], start=(c == 0),
                                     stop=(c == KC - 1))
                dummy = small.tile([128, 1], f32, name=f"dm{j}_{t}", tag="dm")
                nc.vector.affine_mul_reduce(
                    out=dummy.broadcast_to(pd.shape),
                    accum_out=mcst[t][:, j:j + 1],
                    in0=pd, in1=mfss_store[j][t], scale=rx[t], bias=0.0)

            pT = ps_s.tile([1, MSUB], f32, name=f"pT{j}", tag="pS")
            for c in range(KC):
                nc.tensor.matmul(pT, wcol[:, c:c + 1], yt4[:, c, :],
                                 start=(c == 0), stop=(c == KC - 1))
            sT = evac_pool.tile([1, MSUB], f32, name=f"sT{j}", tag="sS")
            nc.scalar.activation(sT, pT, AF.Copy)
            nc.sync.dma_start(out=T_out[j * MSUB:(j + 1) * MSUB], in_=sT)

        nc.sync.dma_start(out=ry_out[:], in_=ry_dram[:])

        # mc row sums
        mc_row = singles.tile([128, NT], f32)
        for t in range(NT):
            nc.vector.tensor_reduce(out=mc_row[:, t:t + 1], in_=mcst[t], axis=X,
                                    op=OP.add)
        nc.sync.dma_start(out=mc_out.rearrange("(t p) -> p t", p=128), in_=mc_row)

    nc.finalize()
    return nc


def _get_nc():
    if "nc" not in _cached:
        _cached["nc"] = _build_nc()
    return _cached["nc"]


def _mk_xpts(xp):
    x0h, x0l = _split11(xp[:, 0])
    x1h, x1l = _split11(xp[:, 1])
    one = np.ones(xp.shape[0], np.float32)
    # row k of xpts pairs with row k of ypts: [y0h,y0l,y0h,y0l,y1h,y1l,y1h,y1l,yy1,yy2,yy3]
    return np.ascontiguousarray(np.stack(
        [-2 * x0h, -2 * x0h, -2 * x0l, -2 * x0l,
         -2 * x1h, -2 * x1h, -2 * x1l, -2 * x1l, one, one, one]))


def _bf16np():
    import ml_dtypes
    return ml_dtypes.bfloat16


def _make_in_maps(valid_pts_scr, mem_pts_scr, valid_desc, mem_desc):
    y0h, y0l = _split11(mem_pts_scr[:, 0])
    y1h, y1l = _split11(mem_pts_scr[:, 1])
    yy64 = (mem_pts_scr[:, 0].astype(np.float64) ** 2
            + mem_pts_scr[:, 1].astype(np.float64) ** 2)
    yy1, yy2, yy3 = _split11_multi(yy64, 3)
    ypts = np.ascontiguousarray(
        np.stack([y0h, y0l, y0h, y0l, y1h, y1l, y1h, y1l, yy1, yy2, yy3]))
    import ml_dtypes
    yT = np.ascontiguousarray(mem_desc.T.astype(ml_dtypes.bfloat16))

    in_maps = []
    for c in range(NCORES):
        sl = slice(c * NL, (c + 1) * NL)
        xs = valid_desc[sl]
        xp = valid_pts_scr[sl]
        in_maps.append({
            "xdT": np.ascontiguousarray(xs.T.astype(_bf16np())),
            "x_nat": np.ascontiguousarray(xs),
            "xpts": _mk_xpts(xp),
            "thr": np.ascontiguousarray(
                (4.0 - xp[:, 0].astype(np.float64) ** 2
                 - xp[:, 1].astype(np.float64) ** 2).astype(np.float32)),
            "ypts": ypts,
            "yT": yT,
        })
    return in_maps


def _finish(results):
    S = np.zeros(M, np.float64)
    T = np.zeros(M, np.float64)
    A = 0.0
    nrows = 0.0
    for c in range(NCORES):
        r = results[c]
        S += r["S_out"].astype(np.float64)
        T += r["T_out"].astype(np.float64) * r["ry_out"].astype(np.float64)
        rc = r["rc_out"].astype(np.float64)
        mc = r["mc_out"].astype(np.float64)
        rh = rc > 0
        A += float(((rc - 2.0 * mc) * rh).sum())
        nrows += float(rh.sum())
    npairs = float(S.sum())
    if nrows > 0:
        loss = (float(S @ T) + A) / (max(npairs, 1.0) * max(nrows, 1.0))
    else:
        loss = 0.0
    return np.float32(loss)


def kernel(valid_pts_scr, mem_pts_scr, valid_desc, mem_desc):
    from concourse.bass_utils import run_bass_kernel_spmd

    in_maps = _make_in_maps(
        np.asarray(valid_pts_scr, dtype=np.float32),
        np.asarray(mem_pts_scr, dtype=np.float32),
        np.asarray(valid_desc, dtype=np.float32),
        np.asarray(mem_desc, dtype=np.float32))

    nc = _get_nc()
    res = run_bass_kernel_spmd(nc, in_maps, core_ids=list(range(NCORES)))
    _cached["last_results"] = res
    return _finish(res.results)
